# revision 19
# baseline (speedup 1.0000x reference)
"""GQA kernel for trn2, 8 NeuronCores.

Sharding: DP over batch (2) x TP over heads (4 groups):
core c -> batch bi=c//4, head-group g=c%4 (q-heads 8g..8g+7, kv-heads
2g,2g+1, wq/wk/wv column-slices, wo row-slice).

Wire traffic is minimized (the axon tunnel is a shared ~50MB/s pipe, so
end-to-end latency is transfer-bound): x crosses as int8 with per-(128-
T-block, D-column) absmax scales in fp16; weights cross as fp16; each
core uploads only a T-quarter of x^T (AllGather over the 4 cores of its
batch rebuilds the full x^T on device) and only half of its weight
slices (AllGather over the batch-pair rebuilds them); the per-core
partial outputs are ReduceScattered on device and quantized to int8
with per-T-row absmax scales, so each core downloads just 1MB. Every
tensor byte crosses the tunnel exactly once (~28MB up, ~8.4MB down).
Constants (ones/zeros layout for V) are memset on device; output
buffers are donated device-built zeros; host packing overlaps the
async uploads; the jitted runner is cached across calls.

On-core compute (all matmuls fp16 with f32 PSUM accumulation):
Q^T/K^T/V^T via matmul with weights stationary; attention in S^T layout
(k on partitions) so no transposes are needed except V (tiny 128x128
TensorE transposes); softmax normalization folded as a 1/rowsum multiply
on the attention output; final projection contracts the per-core 512
head-cols against the wo row-slice into a [T, D] partial that the
ReduceScatter sums.
"""
import sys
sys.path.insert(0, '/opt/trn_rl_repo')
import numpy as np

B, T, D = 2, 2048, 2048
HEADS_PER_CORE = 8      # q heads per core
KV_PER_CORE = 2
DH = 64
SCALE = 0.125           # 1/sqrt(64)
NQB = 4                 # q blocks of 512
NTQ = 4                 # T quarters for projection streaming
KIN = 16                # contraction tiles over D
NCORES = 8

G4 = [[0, 1, 2, 3], [4, 5, 6, 7]]          # the 4 cores of one batch
G2 = [[0, 4], [1, 5], [2, 6], [3, 7]]      # batch-pair (same head group)

_cache = {}


def _build():
    if "nc" in _cache:
        return _cache["nc"]
    import concourse.bass as bass
    from concourse import bacc, mybir
    import concourse.tile as tile
    from concourse.masks import make_identity

    f32 = mybir.dt.float32
    f32r = mybir.dt.float32r
    f16 = mybir.dt.float16
    i8 = mybir.dt.int8
    AF = mybir.ActivationFunctionType
    ADD = mybir.AluOpType.add
    BYP = mybir.AluOpType.bypass

    nc = bacc.Bacc(num_devices=NCORES)
    # per-core uploads: T-quarter of x^T as int8 (scales per (128-T-block,
    # D-column) in fp16), half of each weight slice in fp16
    xq8 = nc.declare_dram_parameter("xq8", [D, 512], i8, isOutput=False)
    xscl = nc.declare_dram_parameter("xscl", [16, D], f16, isOutput=False)
    wqh = nc.declare_dram_parameter("wqh", [1024, 512], f16, isOutput=False)
    wkh = nc.declare_dram_parameter("wkh", [1024, 128], f16, isOutput=False)
    wvh = nc.declare_dram_parameter("wvh", [1024, 128], f16, isOutput=False)
    woh = nc.declare_dram_parameter("woh", [256, D], f16, isOutput=False)
    # output: per-T-row int8 with f32 row-absmax (host divides by 127)
    out8 = nc.declare_dram_parameter("out8", [512, D], i8, isOutput=True)
    oscl = nc.declare_dram_parameter("oscl", [512], f32, isOutput=True)

    with tile.TileContext(nc) as tc:
        with tc.tile_pool(name="dram", bufs=1, space="DRAM") as dram, \
             tc.tile_pool(name="wbig", bufs=1) as wbig, \
             tc.tile_pool(name="wsmall", bufs=1) as wsmall, \
             tc.tile_pool(name="persist", bufs=1) as persist, \
             tc.tile_pool(name="xtp", bufs=6) as xtp, \
             tc.tile_pool(name="exps", bufs=4) as exps, \
             tc.tile_pool(name="small", bufs=4) as small, \
             tc.tile_pool(name="yout", bufs=3) as yout:

            # ---- DRAM scratch: collective bounce buffers ----
            bx = dram.tile([D, 512], i8)
            bwq = dram.tile([1024, 512], f16)
            bwk = dram.tile([1024, 128], f16)
            bwv = dram.tile([1024, 128], f16)
            bwo = dram.tile([256, D], f16)
            xg = dram.tile([4, D, 512], i8)       # gathered x^T (quarter j = T cols 512j..)
            wqg = dram.tile([D, 512], f16)
            wkg = dram.tile([D, 128], f16)
            wvg = dram.tile([D, 128], f16)
            wog = dram.tile([512, D], f16)
            ypart = dram.tile([T, D], f16)        # this core's output partial
            yred = dram.tile([512, D], f16)       # reduce-scattered slice

            # x first: the projection stream is the critical-path start
            nc.gpsimd.dma_start(bx[:], xq8[:])
            nc.gpsimd.collective_compute("AllGather", BYP, replica_groups=G4,
                                         ins=[bx.opt()], outs=[xg.opt()])
            nc.gpsimd.dma_start(bwq[:], wqh[:])
            nc.gpsimd.collective_compute("AllGather", BYP, replica_groups=G2,
                                         ins=[bwq.opt()], outs=[wqg.opt()])
            nc.gpsimd.dma_start(bwk[:], wkh[:])
            nc.gpsimd.collective_compute("AllGather", BYP, replica_groups=G2,
                                         ins=[bwk.opt()], outs=[wkg.opt()])
            nc.gpsimd.dma_start(bwv[:], wvh[:])
            nc.gpsimd.collective_compute("AllGather", BYP, replica_groups=G2,
                                         ins=[bwv.opt()], outs=[wvg.opt()])
            nc.gpsimd.dma_start(bwo[:], woh[:])
            nc.gpsimd.collective_compute("AllGather", BYP, replica_groups=G2,
                                         ins=[bwo.opt()], outs=[wog.opt()])

            # ---- resident weights (fp16) ----
            wq_sb = wbig.tile([128, KIN, 512], f16, tag="wq")
            wo_sb = wbig.tile([128, 4, T], f16, tag="wo")
            wk_sb = wsmall.tile([128, KIN, 128], f16, tag="wk")
            wv_sb = wsmall.tile([128, KIN, 128], f16, tag="wv")
            for kin in range(KIN):
                rs_ = slice(kin * 128, (kin + 1) * 128)
                nc.sync.dma_start(out=wq_sb[:, kin, :], in_=wqg[rs_, :])
                nc.sync.dma_start(out=wk_sb[:, kin, :], in_=wkg[rs_, :])
                nc.sync.dma_start(out=wv_sb[:, kin, :], in_=wvg[rs_, :])
            for c in range(4):
                nc.sync.dma_start(out=wo_sb[:, c, :], in_=wog[c * 128:(c + 1) * 128, :])

            # x dequant scales: [16 tblocks, D] fp16 -> [128, KIN, 16] f32
            xscl_r = xscl.rearrange("tb (kin p) -> kin p tb", p=128)
            s16_sb = persist.tile([128, KIN, 16], f16)
            for kin in range(KIN):
                nc.sync.dma_start(out=s16_sb[:, kin, :], in_=xscl_r[kin])
            scl_sb = persist.tile([128, KIN, 16], f32)
            nc.vector.tensor_copy(out=scl_sb[:], in_=s16_sb[:])

            ident = persist.tile([128, 128], f32)
            make_identity(nc, ident)
            ones32 = persist.tile([128, 128], f32)
            nc.gpsimd.memset(ones32[:], 1.0)
            ones_sb = persist.tile([128, 128], f32r)
            nc.vector.tensor_copy(out=ones_sb[:], in_=ones32[:])

            # ---- persistent activations ----
            # QT: 4 chunks of [128, T] (q head-cols on partitions)
            qt_sb = persist.tile([128, 4, T], f16)
            # KT: [128, T]; rows 0-63 = kv0 K^T, 64-127 = kv1 K^T
            kt_sb = persist.tile([128, T], f16)
            # V natural layout + ones col: per kv head, 16 tiles.
            # kv0: cols 0-63 = V, col 64 = ones  -> O at partitions 0-63, sums at 64
            # kv1: col 0 = ones, cols 64-127 = V -> sums at partition 0, O at 64-127
            v_sb = persist.tile([128, KV_PER_CORE, 16, 128], f16)
            nc.gpsimd.memset(v_sb[:], 0.0)
            nc.gpsimd.memset(v_sb[:, 0, :, 64:65], 1.0)
            nc.gpsimd.memset(v_sb[:, 1, :, 0:1], 1.0)
            # attention out (pre-wo), lhsT layout: 4 chunks [128, T]
            ot_sb = persist.tile([128, 4, T], f16)

            # ---- phase B: projections (stream x^T in T-quarters) ----
            pb = tc.tile_pool(name="pps", bufs=6, space="PSUM")
            pps = pb.__enter__()
            tb = tc.tile_pool(name="tps", bufs=2, space="PSUM")
            tps = tb.__enter__()
            for tq in range(NTQ):
                ts_ = slice(tq * 512, (tq + 1) * 512)
                qps = []
                for mc in range(4):
                    qp_t = pps.tile([128, 512], f32, tag="ps")
                    qps.append(qp_t)
                kps = pps.tile([128, 512], f32, tag="ps")
                vps = pps.tile([128, 512], f32, tag="ps")
                for kin in range(KIN):
                    xt8 = xtp.tile([128, 512], i8, tag="xt8")
                    nc.sync.dma_start(out=xt8, in_=xg[tq, kin * 128:(kin + 1) * 128, :])
                    xtile = xtp.tile([128, 512], f16, tag="xt")
                    for dq4 in range(4):
                        nc.vector.tensor_scalar_mul(
                            xtile[:, dq4 * 128:(dq4 + 1) * 128],
                            xt8[:, dq4 * 128:(dq4 + 1) * 128],
                            scl_sb[:, kin, tq * 4 + dq4:tq * 4 + dq4 + 1])
                    st, sp = (kin == 0), (kin == KIN - 1)
                    for mc in range(4):
                        nc.tensor.matmul(qps[mc], wq_sb[:, kin, mc * 128:(mc + 1) * 128],
                                         xtile, start=st, stop=sp)
                    nc.tensor.matmul(kps, wk_sb[:, kin, :], xtile, start=st, stop=sp)
                    nc.tensor.matmul(vps, wv_sb[:, kin, :], xtile, start=st, stop=sp)
                for mc in range(4):
                    nc.vector.tensor_copy(out=qt_sb[:, mc, ts_], in_=qps[mc])
                nc.vector.tensor_copy(out=kt_sb[:, ts_], in_=kps)
                # V^T chunk -> transpose to natural V tiles
                vt_sb = small.tile([128, 512], f32, tag="vt")
                nc.vector.tensor_copy(out=vt_sb, in_=vps)
                for st4 in range(4):
                    tt = tq * 4 + st4
                    trp = tps.tile([128, 128], f32, tag="tp")
                    nc.tensor.transpose(trp, vt_sb[:, st4 * 128:(st4 + 1) * 128], ident)
                    nc.vector.tensor_copy(out=v_sb[:, 0, tt, 0:64], in_=trp[:, 0:64])
                    nc.vector.tensor_copy(out=v_sb[:, 1, tt, 64:128], in_=trp[:, 64:128])

            tb.__exit__(None, None, None)
            pb.__exit__(None, None, None)

            # ---- phase C+D fused: attention (qb outer) + output proj per q-block ----
            sb_ = tc.tile_pool(name="spp", bufs=5, space="PSUM")
            spp = sb_.__enter__()
            ob_ = tc.tile_pool(name="opp", bufs=3, space="PSUM")
            opp = ob_.__enter__()
            for qb in range(NQB):
                qs = slice(qb * 512, (qb + 1) * 512)
                nkt = 4 * (qb + 1)
                for h in range(HEADS_PER_CORE):
                    kv = h // 4
                    mc = h % 4          # host packs head h with head h+4 in chunk h%4
                    row0 = 64 * kv      # h<4 at partitions 0-63, h>=4 at 64-127
                    q_rows = slice(row0, row0 + 64)
                    k_rows = slice(row0, row0 + 64)
                    o_ps = opp.tile([128, 512], f32, tag="op")
                    prev = None
                    for kt in range(nkt):
                        s_ps = spp.tile([128, 512], f32, tag="sp")
                        nc.tensor.matmul(s_ps,
                                         kt_sb[k_rows, kt * 128:(kt + 1) * 128],
                                         qt_sb[q_rows, mc, qs],
                                         start=True, stop=True)
                        e_sb = exps.tile([128, 512], f16, tag="ex")
                        nc.scalar.activation(out=e_sb, in_=s_ps, func=AF.Exp, scale=SCALE)
                        if kt >= 4 * qb:
                            nc.gpsimd.affine_select(
                                out=e_sb, in_=e_sb,
                                pattern=[[1, 512]],
                                compare_op=mybir.AluOpType.is_ge,
                                fill=0.0,
                                base=-128 * (kt - 4 * qb),
                                channel_multiplier=-1)
                        # software-pipeline the PV matmul one step behind
                        if prev is not None:
                            pkt, pe = prev
                            vl = v_sb[:, 0, pkt, 0:65] if kv == 0 else v_sb[:, 1, pkt, :]
                            nc.tensor.matmul(o_ps[0:65, :] if kv == 0 else o_ps,
                                             vl, pe, start=(pkt == 0), stop=False)
                        prev = (kt, e_sb)
                    pkt, pe = prev
                    vl = v_sb[:, 0, pkt, 0:65] if kv == 0 else v_sb[:, 1, pkt, :]
                    nc.tensor.matmul(o_ps[0:65, :] if kv == 0 else o_ps,
                                     vl, pe, start=(pkt == 0), stop=True)
                    # normalize: O rows / sums row (layout depends on kv)
                    srow = slice(64, 65) if kv == 0 else slice(0, 1)
                    orow = slice(0, 64) if kv == 0 else slice(64, 128)
                    r_sb = small.tile([128, 512], f32r, tag="r")
                    with nc.allow_low_precision(reason="f32r reciprocal for matmul rhs"):
                        nc.vector.reciprocal(out=r_sb[srow, :], in_=o_ps[srow, :])
                    # broadcast r across partitions: ones[1,128].T @ r[1,512]
                    ob0 = 64 - row0   # partition where the sums row lives
                    ones_row = ones_sb[ob0:ob0 + 1, 0:128]
                    rb_ps = spp.tile([128, 512], f32, tag="sp")
                    nc.tensor.matmul(rb_ps, ones_row, r_sb[srow, :],
                                     start=True, stop=True)
                    rb_sb = small.tile([128, 512], f32, tag="rb")
                    nc.vector.tensor_copy(out=rb_sb[orow, :], in_=rb_ps[orow, :])
                    nc.vector.tensor_tensor(
                        out=ot_sb[q_rows, mc, qs],
                        in0=o_ps[orow, :], in1=rb_sb[orow, :],
                        op=mybir.AluOpType.mult)
                # output projection for this q-block (overlaps next qb's attention)
                for tt in range(4 * qb, 4 * qb + 4):
                    tsl = slice(tt * 128, (tt + 1) * 128)
                    for nb in range(4):
                        nsl = slice(nb * 512, (nb + 1) * 512)
                        y_ps = opp.tile([128, 512], f32, tag="op")
                        for c in range(4):
                            nc.tensor.matmul(y_ps, ot_sb[:, c, tsl], wo_sb[:, c, nsl],
                                             start=(c == 0), stop=(c == 3))
                        y_sb = yout.tile([128, 512], f16, tag="y")
                        if (tt * 4 + nb) % 2 == 0:
                            nc.vector.tensor_copy(out=y_sb, in_=y_ps)
                        else:
                            nc.scalar.activation(out=y_sb, in_=y_ps, func=AF.Copy)
                        nc.sync.dma_start(out=ypart[tsl, nsl], in_=y_sb)
            ob_.__exit__(None, None, None)
            sb_.__exit__(None, None, None)

            # ---- on-device cross-core reduction of the output partials ----
            nc.gpsimd.collective_compute("ReduceScatter", ADD, replica_groups=G4,
                                         ins=[ypart.opt()], outs=[yred.opt()])
            # quantize the reduced slice to int8 with per-row absmax scales
            qb_ = tc.tile_pool(name="oq", bufs=2)
            oqp = qb_.__enter__()
            for c4 in range(4):
                rsl = slice(c4 * 128, (c4 + 1) * 128)
                ytile = oqp.tile([128, D], f16, tag="yt")
                nc.sync.dma_start(out=ytile, in_=yred[rsl, :])
                mx = oqp.tile([128, 1], f32, tag="mx")
                nc.vector.tensor_reduce(out=mx, in_=ytile, axis=mybir.AxisListType.X,
                                        op=mybir.AluOpType.max,
                                        apply_absolute_value=True)
                nc.vector.tensor_scalar_max(mx, mx, 1e-30)
                r_ = oqp.tile([128, 1], f32, tag="r")
                nc.vector.reciprocal(out=r_, in_=mx)
                q8 = oqp.tile([128, D], i8, tag="q8")
                nc.vector.tensor_scalar(out=q8, in0=ytile, scalar1=r_[:, 0:1],
                                        scalar2=127.0,
                                        op0=mybir.AluOpType.mult,
                                        op1=mybir.AluOpType.mult)
                nc.sync.dma_start(out=out8[rsl, :], in_=q8)
                nc.sync.dma_start(out=oscl[rsl], in_=mx[:, 0])
            qb_.__exit__(None, None, None)

    nc.finalize()
    _cache["nc"] = nc
    return nc


def _get_runner():
    if "runner" in _cache:
        return _cache["runner"]
    import jax
    import jax.numpy as jnp
    from jax.sharding import Mesh, NamedSharding, PartitionSpec as P
    import warnings
    with warnings.catch_warnings():
        warnings.simplefilter("ignore")
        try:
            from jax.experimental.shard_map import shard_map
        except ImportError:
            from jax import shard_map
    from concourse import mybir
    from concourse.bass2jax import (_bass_exec_p, install_neuronx_cc_hook,
                                    partition_id_tensor)

    nc = _build()
    install_neuronx_cc_hook()
    partition_name = nc.partition_id_tensor.name if nc.partition_id_tensor else None
    in_names, out_names, out_avals = [], [], []
    for alloc in nc.m.functions[0].allocations:
        if not isinstance(alloc, mybir.MemoryLocationSet):
            continue
        name = alloc.memorylocations[0].name
        if alloc.kind == "ExternalInput":
            if name != partition_name:
                in_names.append(name)
        elif alloc.kind == "ExternalOutput":
            out_names.append(name)
            out_avals.append(jax.core.ShapedArray(
                tuple(alloc.tensor_shape), mybir.dt.np(alloc.dtype)))
    n_params = len(in_names)
    n_outs = len(out_names)
    in_names_full = in_names + out_names
    if partition_name is not None:
        in_names_full.append(partition_name)
    donate = tuple(range(n_params, n_params + n_outs))

    def _body(*args):
        operands = list(args)
        if partition_name is not None:
            operands.append(partition_id_tensor())
        outs = _bass_exec_p.bind(
            *operands, out_avals=tuple(out_avals), in_names=tuple(in_names_full),
            out_names=tuple(out_names), lowering_input_output_aliases=(),
            sim_require_finite=True, sim_require_nnan=True, nc=nc)
        return tuple(outs)

    devices = jax.devices()[:NCORES]
    mesh = Mesh(np.asarray(devices), ("core",))
    sharded = jax.jit(
        shard_map(_body, mesh=mesh, in_specs=(P("core"),) * (n_params + n_outs),
                  out_specs=(P("core"),) * n_outs, check_rep=False),
        donate_argnums=donate, keep_unused=True)
    zsh = NamedSharding(mesh, P("core"))
    # output buffers are donated zeros; build them on-device (no host upload)
    zeros_fn = jax.jit(
        lambda: tuple(jnp.zeros((NCORES * a.shape[0], *a.shape[1:]), a.dtype)
                      for a in out_avals),
        out_shardings=(zsh,) * n_outs)
    _cache["runner"] = (sharded, zeros_fn, in_names, out_names, zsh)
    return _cache["runner"]


_HEAD_ORDER = [0, 4, 1, 5, 2, 6, 3, 7]


def kernel(x, wq, wk, wv, wo, attention_mask=None, **_ignored):
    import jax
    from concurrent.futures import ThreadPoolExecutor
    sharded, zeros_fn, in_names, out_names, zsh = _get_runner()
    zs = zeros_fn()  # async: device-side zero buffers materialize in background

    # upload order = pack speed: wk/wv pack fastest, so their puts start
    # filling the (serial) tunnel while the bigger tensors are packed.
    # All weight slices are halved over the batch-pair (bi=0 -> first
    # half, bi=1 -> second half); wq/wo are permuted so head h packs
    # with h+4 (pairs (i, i+4) == reshape(2,4,..).swapaxes trick).
    wk16 = np.asarray(wk, dtype=np.float16)
    wv16 = np.asarray(wv, dtype=np.float16)
    cwkh = np.empty((NCORES * 1024, 128), np.float16)
    cwvh = np.empty((NCORES * 1024, 128), np.float16)
    for c in range(NCORES):
        bi, g = c // 4, c % 4
        cwkh[c * 1024:(c + 1) * 1024] = wk16[1024 * bi:1024 * (bi + 1),
                                             128 * g:128 * (g + 1)]
        cwvh[c * 1024:(c + 1) * 1024] = wv16[1024 * bi:1024 * (bi + 1),
                                             128 * g:128 * (g + 1)]
    dev = {"wkh": jax.device_put(cwkh, zsh), "wvh": jax.device_put(cwvh, zsh)}

    wq16 = np.asarray(wq, dtype=np.float16)
    cwqh = np.empty((NCORES * 1024, 512), np.float16)
    for g in range(4):
        wq_g = wq16[:, 512 * g:512 * (g + 1)].reshape(D, 2, 4, DH)
        wq_g = wq_g.swapaxes(1, 2).reshape(D, 512)
        cwqh[g * 1024:(g + 1) * 1024] = wq_g[:1024]
        cwqh[(4 + g) * 1024:(5 + g) * 1024] = wq_g[1024:]
    dev["wqh"] = jax.device_put(cwqh, zsh)

    # x: int8 with per-(128-T-block, D-column) absmax scales, then the
    # transposed T-quarter for each core's gather position
    xf = np.asarray(x, dtype=np.float32)
    cxq8 = np.empty((NCORES * D, 512), np.int8)
    cxscl = np.empty((NCORES * 16, D), np.float16)
    for bi in range(B):
        xb = xf[bi].reshape(16, 128, D)
        mx = np.abs(xb).max(axis=1)                      # [16, D]
        np.maximum(mx, 1e-30, out=mx)
        scl16 = (mx * (1.0 / 127.0)).astype(np.float16)  # uploaded scales
        inv = 127.0 / mx
        q8 = np.round(xb * inv[:, None, :]).astype(np.int8).reshape(T, D)
        for g in range(4):
            c = bi * 4 + g
            cxq8[c * D:(c + 1) * D] = q8[512 * g:512 * (g + 1), :].T
            cxscl[c * 16:(c + 1) * 16] = scl16
    dev["xq8"] = jax.device_put(cxq8, zsh)
    dev["xscl"] = jax.device_put(cxscl, zsh)

    wo16 = np.asarray(wo, dtype=np.float16)
    cwoh = np.empty((NCORES * 256, D), np.float16)
    for g in range(4):
        wo_g = wo16[512 * g:512 * (g + 1), :].reshape(2, 4, DH, D)
        wo_g = wo_g.swapaxes(0, 1).reshape(512, D)
        cwoh[g * 256:(g + 1) * 256] = wo_g[:256]
        cwoh[(4 + g) * 256:(5 + g) * 256] = wo_g[256:]
    dev["woh"] = jax.device_put(cwoh, zsh)

    out_arrs = sharded(*[dev[n] for n in in_names], *zs)
    # core order: (b0 q0..q3, b1 q0..q3), each [512, D] -> [B, T, D]
    with ThreadPoolExecutor(2) as ex:
        futs = [ex.submit(np.asarray, a) for a in out_arrs]
        outs = {n: f.result() for n, f in zip(out_names, futs)}
    scl = outs["oscl"].astype(np.float32) * (1.0 / 127.0)
    y = outs["out8"].astype(np.float32) * scl[:, None]
    return y.reshape(B, T, D)


# revision 20
# speedup vs baseline: 1.0673x; 1.0673x over previous
"""GQA kernel for trn2, 8 NeuronCores.

Sharding: DP over batch (2) x TP over heads (4 groups):
core c -> batch bi=c//4, head-group g=c%4 (q-heads 8g..8g+7, kv-heads
2g,2g+1, wq/wk/wv column-slices, wo row-slice).

Wire traffic is minimized (the axon tunnel is a shared ~50MB/s pipe, so
end-to-end latency is transfer-bound): x crosses as int8 with per-(128-
T-block, D-column) absmax scales in fp16; weights cross as fp16; each
core uploads only a T-quarter of x^T (AllGather over the 4 cores of its
batch rebuilds the full x^T on device) and only half of its weight
slices (AllGather over the batch-pair rebuilds them); the per-core
partial outputs are ReduceScattered on device and quantized to int8
with per-T-row absmax scales, so each core downloads just 1MB. Every
tensor byte crosses the tunnel exactly once (~28MB up, ~8.4MB down).
Constants (ones/zeros layout for V) are memset on device; output
buffers are donated device-built zeros; host packing overlaps the
async uploads; the jitted runner is cached across calls.

On-core compute (all matmuls fp16 with f32 PSUM accumulation):
Q^T/K^T/V^T via matmul with weights stationary; attention in S^T layout
(k on partitions) so no transposes are needed except V (tiny 128x128
TensorE transposes); softmax normalization folded as a 1/rowsum multiply
on the attention output; final projection contracts the per-core 512
head-cols against the wo row-slice into a [T, D] partial that the
ReduceScatter sums.
"""
import sys
sys.path.insert(0, '/opt/trn_rl_repo')
import numpy as np

B, T, D = 2, 2048, 2048
HEADS_PER_CORE = 8      # q heads per core
KV_PER_CORE = 2
DH = 64
SCALE = 0.125           # 1/sqrt(64)
NQB = 4                 # q blocks of 512
NTQ = 4                 # T quarters for projection streaming
KIN = 16                # contraction tiles over D
NCORES = 8

G4 = [[0, 1, 2, 3], [4, 5, 6, 7]]          # the 4 cores of one batch
G2 = [[0, 4], [1, 5], [2, 6], [3, 7]]      # batch-pair (same head group)

_cache = {}


def _build():
    if "nc" in _cache:
        return _cache["nc"]
    import concourse.bass as bass
    from concourse import bacc, mybir
    import concourse.tile as tile
    from concourse.masks import make_identity

    f32 = mybir.dt.float32
    f32r = mybir.dt.float32r
    f16 = mybir.dt.float16
    i8 = mybir.dt.int8
    AF = mybir.ActivationFunctionType
    ADD = mybir.AluOpType.add
    BYP = mybir.AluOpType.bypass

    nc = bacc.Bacc(num_devices=NCORES)
    # per-core uploads: T-quarter of x^T as int8 (scales per (128-T-block,
    # D-column) in fp16), half of each weight slice in fp16
    xq8 = nc.declare_dram_parameter("xq8", [D, 512], i8, isOutput=False)
    xscl = nc.declare_dram_parameter("xscl", [16, D], f16, isOutput=False)
    wqh = nc.declare_dram_parameter("wqh", [1024, 512], f16, isOutput=False)
    wkh = nc.declare_dram_parameter("wkh", [1024, 128], f16, isOutput=False)
    wvh = nc.declare_dram_parameter("wvh", [1024, 128], f16, isOutput=False)
    woh = nc.declare_dram_parameter("woh", [256, D], f16, isOutput=False)
    # output: per-T-row int8 with f32 row-absmax (host divides by 127)
    out8 = nc.declare_dram_parameter("out8", [512, D], i8, isOutput=True)
    oscl = nc.declare_dram_parameter("oscl", [512], f32, isOutput=True)

    with tile.TileContext(nc) as tc:
        with tc.tile_pool(name="dram", bufs=1, space="DRAM") as dram, \
             tc.tile_pool(name="wbig", bufs=1) as wbig, \
             tc.tile_pool(name="wsmall", bufs=1) as wsmall, \
             tc.tile_pool(name="persist", bufs=1) as persist, \
             tc.tile_pool(name="xtp", bufs=6) as xtp, \
             tc.tile_pool(name="exps", bufs=4) as exps, \
             tc.tile_pool(name="small", bufs=4) as small, \
             tc.tile_pool(name="yout", bufs=3) as yout:

            # ---- DRAM scratch: collective bounce buffers ----
            bx = dram.tile([D, 512], i8)
            bwq = dram.tile([1024, 512], f16)
            bwk = dram.tile([1024, 128], f16)
            bwv = dram.tile([1024, 128], f16)
            bwo = dram.tile([256, D], f16)
            xg = dram.tile([4, D, 512], i8)       # gathered x^T (quarter j = T cols 512j..)
            wqg = dram.tile([D, 512], f16)
            wkg = dram.tile([D, 128], f16)
            wvg = dram.tile([D, 128], f16)
            wog = dram.tile([512, D], f16)
            ypart = dram.tile([T, D], f16)        # this core's output partial
            yred = dram.tile([512, D], f16)       # reduce-scattered slice

            # x first: the projection stream is the critical-path start
            nc.gpsimd.dma_start(bx[:], xq8[:])
            nc.gpsimd.collective_compute("AllGather", BYP, replica_groups=G4,
                                         ins=[bx.opt()], outs=[xg.opt()])
            nc.gpsimd.dma_start(bwq[:], wqh[:])
            nc.gpsimd.collective_compute("AllGather", BYP, replica_groups=G2,
                                         ins=[bwq.opt()], outs=[wqg.opt()])
            nc.gpsimd.dma_start(bwk[:], wkh[:])
            nc.gpsimd.collective_compute("AllGather", BYP, replica_groups=G2,
                                         ins=[bwk.opt()], outs=[wkg.opt()])
            nc.gpsimd.dma_start(bwv[:], wvh[:])
            nc.gpsimd.collective_compute("AllGather", BYP, replica_groups=G2,
                                         ins=[bwv.opt()], outs=[wvg.opt()])
            nc.gpsimd.dma_start(bwo[:], woh[:])
            nc.gpsimd.collective_compute("AllGather", BYP, replica_groups=G2,
                                         ins=[bwo.opt()], outs=[wog.opt()])

            # ---- resident weights (fp16) ----
            wq_sb = wbig.tile([128, KIN, 512], f16, tag="wq")
            wo_sb = wbig.tile([128, 4, T], f16, tag="wo")
            wk_sb = wsmall.tile([128, KIN, 128], f16, tag="wk")
            wv_sb = wsmall.tile([128, KIN, 128], f16, tag="wv")
            for kin in range(KIN):
                rs_ = slice(kin * 128, (kin + 1) * 128)
                nc.sync.dma_start(out=wq_sb[:, kin, :], in_=wqg[rs_, :])
                nc.sync.dma_start(out=wk_sb[:, kin, :], in_=wkg[rs_, :])
                nc.sync.dma_start(out=wv_sb[:, kin, :], in_=wvg[rs_, :])
            for c in range(4):
                nc.sync.dma_start(out=wo_sb[:, c, :], in_=wog[c * 128:(c + 1) * 128, :])

            # x dequant scales: [16 tblocks, D] fp16 -> [128, KIN, 16] f32
            xscl_r = xscl.rearrange("tb (kin p) -> kin p tb", p=128)
            s16_sb = persist.tile([128, KIN, 16], f16)
            for kin in range(KIN):
                nc.sync.dma_start(out=s16_sb[:, kin, :], in_=xscl_r[kin])
            scl_sb = persist.tile([128, KIN, 16], f32)
            nc.vector.tensor_copy(out=scl_sb[:], in_=s16_sb[:])

            ident = persist.tile([128, 128], f32)
            make_identity(nc, ident)
            ones32 = persist.tile([128, 128], f32)
            nc.gpsimd.memset(ones32[:], 1.0)
            ones_sb = persist.tile([128, 128], f32r)
            nc.vector.tensor_copy(out=ones_sb[:], in_=ones32[:])

            # ---- persistent activations ----
            # QT: 4 chunks of [128, T] (q head-cols on partitions)
            qt_sb = persist.tile([128, 4, T], f16)
            # KT: [128, T]; rows 0-63 = kv0 K^T, 64-127 = kv1 K^T
            kt_sb = persist.tile([128, T], f16)
            # V natural layout + ones col: per kv head, 16 tiles.
            # kv0: cols 0-63 = V, col 64 = ones  -> O at partitions 0-63, sums at 64
            # kv1: col 0 = ones, cols 64-127 = V -> sums at partition 0, O at 64-127
            v_sb = persist.tile([128, KV_PER_CORE, 16, 128], f16)
            nc.gpsimd.memset(v_sb[:], 0.0)
            nc.gpsimd.memset(v_sb[:, 0, :, 64:65], 1.0)
            nc.gpsimd.memset(v_sb[:, 1, :, 0:1], 1.0)
            # attention out (pre-wo), lhsT layout: 4 chunks [128, T]
            ot_sb = persist.tile([128, 4, T], f16)

            # ---- phase B: projections (stream x^T in T-quarters) ----
            pb = tc.tile_pool(name="pps", bufs=6, space="PSUM")
            pps = pb.__enter__()
            tb = tc.tile_pool(name="tps", bufs=2, space="PSUM")
            tps = tb.__enter__()
            for tq in range(NTQ):
                ts_ = slice(tq * 512, (tq + 1) * 512)
                qps = []
                for mc in range(4):
                    qp_t = pps.tile([128, 512], f32, tag="ps")
                    qps.append(qp_t)
                kps = pps.tile([128, 512], f32, tag="ps")
                vps = pps.tile([128, 512], f32, tag="ps")
                for kin in range(KIN):
                    xt8 = xtp.tile([128, 512], i8, tag="xt8")
                    nc.sync.dma_start(out=xt8, in_=xg[tq, kin * 128:(kin + 1) * 128, :])
                    xtile = xtp.tile([128, 512], f16, tag="xt")
                    for dq4 in range(4):
                        nc.vector.tensor_scalar_mul(
                            xtile[:, dq4 * 128:(dq4 + 1) * 128],
                            xt8[:, dq4 * 128:(dq4 + 1) * 128],
                            scl_sb[:, kin, tq * 4 + dq4:tq * 4 + dq4 + 1])
                    st, sp = (kin == 0), (kin == KIN - 1)
                    for mc in range(4):
                        nc.tensor.matmul(qps[mc], wq_sb[:, kin, mc * 128:(mc + 1) * 128],
                                         xtile, start=st, stop=sp)
                    nc.tensor.matmul(kps, wk_sb[:, kin, :], xtile, start=st, stop=sp)
                    nc.tensor.matmul(vps, wv_sb[:, kin, :], xtile, start=st, stop=sp)
                for mc in range(4):
                    nc.vector.tensor_copy(out=qt_sb[:, mc, ts_], in_=qps[mc])
                nc.vector.tensor_copy(out=kt_sb[:, ts_], in_=kps)
                # V^T chunk -> transpose to natural V tiles
                vt_sb = small.tile([128, 512], f32, tag="vt")
                nc.vector.tensor_copy(out=vt_sb, in_=vps)
                for st4 in range(4):
                    tt = tq * 4 + st4
                    trp = tps.tile([128, 128], f32, tag="tp")
                    nc.tensor.transpose(trp, vt_sb[:, st4 * 128:(st4 + 1) * 128], ident)
                    nc.vector.tensor_copy(out=v_sb[:, 0, tt, 0:64], in_=trp[:, 0:64])
                    nc.vector.tensor_copy(out=v_sb[:, 1, tt, 64:128], in_=trp[:, 64:128])

            tb.__exit__(None, None, None)
            pb.__exit__(None, None, None)

            # ---- phase C+D fused: attention (qb outer) + output proj per q-block ----
            sb_ = tc.tile_pool(name="spp", bufs=5, space="PSUM")
            spp = sb_.__enter__()
            ob_ = tc.tile_pool(name="opp", bufs=3, space="PSUM")
            opp = ob_.__enter__()
            for qb in range(NQB):
                qs = slice(qb * 512, (qb + 1) * 512)
                nkt = 4 * (qb + 1)
                for h in range(HEADS_PER_CORE):
                    kv = h // 4
                    mc = h % 4          # host packs head h with head h+4 in chunk h%4
                    row0 = 64 * kv      # h<4 at partitions 0-63, h>=4 at 64-127
                    q_rows = slice(row0, row0 + 64)
                    k_rows = slice(row0, row0 + 64)
                    o_ps = opp.tile([128, 512], f32, tag="op")
                    prev = None
                    for kt in range(nkt):
                        s_ps = spp.tile([128, 512], f32, tag="sp")
                        nc.tensor.matmul(s_ps,
                                         kt_sb[k_rows, kt * 128:(kt + 1) * 128],
                                         qt_sb[q_rows, mc, qs],
                                         start=True, stop=True)
                        e_sb = exps.tile([128, 512], f16, tag="ex")
                        nc.scalar.activation(out=e_sb, in_=s_ps, func=AF.Exp, scale=SCALE)
                        if kt >= 4 * qb:
                            nc.gpsimd.affine_select(
                                out=e_sb, in_=e_sb,
                                pattern=[[1, 512]],
                                compare_op=mybir.AluOpType.is_ge,
                                fill=0.0,
                                base=-128 * (kt - 4 * qb),
                                channel_multiplier=-1)
                        # software-pipeline the PV matmul one step behind
                        if prev is not None:
                            pkt, pe = prev
                            vl = v_sb[:, 0, pkt, 0:65] if kv == 0 else v_sb[:, 1, pkt, :]
                            nc.tensor.matmul(o_ps[0:65, :] if kv == 0 else o_ps,
                                             vl, pe, start=(pkt == 0), stop=False)
                        prev = (kt, e_sb)
                    pkt, pe = prev
                    vl = v_sb[:, 0, pkt, 0:65] if kv == 0 else v_sb[:, 1, pkt, :]
                    nc.tensor.matmul(o_ps[0:65, :] if kv == 0 else o_ps,
                                     vl, pe, start=(pkt == 0), stop=True)
                    # normalize: O rows / sums row (layout depends on kv)
                    srow = slice(64, 65) if kv == 0 else slice(0, 1)
                    orow = slice(0, 64) if kv == 0 else slice(64, 128)
                    r_sb = small.tile([128, 512], f32r, tag="r")
                    with nc.allow_low_precision(reason="f32r reciprocal for matmul rhs"):
                        nc.vector.reciprocal(out=r_sb[srow, :], in_=o_ps[srow, :])
                    # broadcast r across partitions: ones[1,128].T @ r[1,512]
                    ob0 = 64 - row0   # partition where the sums row lives
                    ones_row = ones_sb[ob0:ob0 + 1, 0:128]
                    rb_ps = spp.tile([128, 512], f32, tag="sp")
                    nc.tensor.matmul(rb_ps, ones_row, r_sb[srow, :],
                                     start=True, stop=True)
                    rb_sb = small.tile([128, 512], f32, tag="rb")
                    nc.vector.tensor_copy(out=rb_sb[orow, :], in_=rb_ps[orow, :])
                    nc.vector.tensor_tensor(
                        out=ot_sb[q_rows, mc, qs],
                        in0=o_ps[orow, :], in1=rb_sb[orow, :],
                        op=mybir.AluOpType.mult)
                # output projection for this q-block (overlaps next qb's attention)
                for tt in range(4 * qb, 4 * qb + 4):
                    tsl = slice(tt * 128, (tt + 1) * 128)
                    for nb in range(4):
                        nsl = slice(nb * 512, (nb + 1) * 512)
                        y_ps = opp.tile([128, 512], f32, tag="op")
                        for c in range(4):
                            nc.tensor.matmul(y_ps, ot_sb[:, c, tsl], wo_sb[:, c, nsl],
                                             start=(c == 0), stop=(c == 3))
                        y_sb = yout.tile([128, 512], f16, tag="y")
                        if (tt * 4 + nb) % 2 == 0:
                            nc.vector.tensor_copy(out=y_sb, in_=y_ps)
                        else:
                            nc.scalar.activation(out=y_sb, in_=y_ps, func=AF.Copy)
                        nc.sync.dma_start(out=ypart[tsl, nsl], in_=y_sb)
            ob_.__exit__(None, None, None)
            sb_.__exit__(None, None, None)

            # ---- on-device cross-core reduction of the output partials ----
            nc.gpsimd.collective_compute("ReduceScatter", ADD, replica_groups=G4,
                                         ins=[ypart.opt()], outs=[yred.opt()])
            # quantize the reduced slice to int8 with per-row absmax scales
            qb_ = tc.tile_pool(name="oq", bufs=2)
            oqp = qb_.__enter__()
            for c4 in range(4):
                rsl = slice(c4 * 128, (c4 + 1) * 128)
                ytile = oqp.tile([128, D], f16, tag="yt")
                nc.sync.dma_start(out=ytile, in_=yred[rsl, :])
                mx = oqp.tile([128, 1], f32, tag="mx")
                nc.vector.tensor_reduce(out=mx, in_=ytile, axis=mybir.AxisListType.X,
                                        op=mybir.AluOpType.max,
                                        apply_absolute_value=True)
                nc.vector.tensor_scalar_max(mx, mx, 1e-30)
                r_ = oqp.tile([128, 1], f32, tag="r")
                nc.vector.reciprocal(out=r_, in_=mx)
                q8 = oqp.tile([128, D], i8, tag="q8")
                nc.vector.tensor_scalar(out=q8, in0=ytile, scalar1=r_[:, 0:1],
                                        scalar2=127.0,
                                        op0=mybir.AluOpType.mult,
                                        op1=mybir.AluOpType.mult)
                nc.sync.dma_start(out=out8[rsl, :], in_=q8)
                nc.sync.dma_start(out=oscl[rsl], in_=mx[:, 0])
            qb_.__exit__(None, None, None)

    nc.finalize()
    _cache["nc"] = nc
    return nc


def _get_runner():
    if "runner" in _cache:
        return _cache["runner"]
    import jax
    import jax.numpy as jnp
    from jax.sharding import Mesh, NamedSharding, PartitionSpec as P
    import warnings
    with warnings.catch_warnings():
        warnings.simplefilter("ignore")
        try:
            from jax.experimental.shard_map import shard_map
        except ImportError:
            from jax import shard_map
    from concourse import mybir
    from concourse.bass2jax import (_bass_exec_p, install_neuronx_cc_hook,
                                    partition_id_tensor)

    nc = _build()
    install_neuronx_cc_hook()
    partition_name = nc.partition_id_tensor.name if nc.partition_id_tensor else None
    in_names, out_names, out_avals = [], [], []
    for alloc in nc.m.functions[0].allocations:
        if not isinstance(alloc, mybir.MemoryLocationSet):
            continue
        name = alloc.memorylocations[0].name
        if alloc.kind == "ExternalInput":
            if name != partition_name:
                in_names.append(name)
        elif alloc.kind == "ExternalOutput":
            out_names.append(name)
            out_avals.append(jax.core.ShapedArray(
                tuple(alloc.tensor_shape), mybir.dt.np(alloc.dtype)))
    n_params = len(in_names)
    n_outs = len(out_names)
    in_names_full = in_names + out_names
    if partition_name is not None:
        in_names_full.append(partition_name)
    donate = tuple(range(n_params, n_params + n_outs))

    def _body(*args):
        operands = list(args)
        if partition_name is not None:
            operands.append(partition_id_tensor())
        outs = _bass_exec_p.bind(
            *operands, out_avals=tuple(out_avals), in_names=tuple(in_names_full),
            out_names=tuple(out_names), lowering_input_output_aliases=(),
            sim_require_finite=True, sim_require_nnan=True, nc=nc)
        return tuple(outs)

    devices = jax.devices()[:NCORES]
    mesh = Mesh(np.asarray(devices), ("core",))
    sharded = jax.jit(
        shard_map(_body, mesh=mesh, in_specs=(P("core"),) * (n_params + n_outs),
                  out_specs=(P("core"),) * n_outs, check_rep=False),
        donate_argnums=donate, keep_unused=True)
    zsh = NamedSharding(mesh, P("core"))
    # output buffers are donated zeros; build them on-device (no host upload)
    zeros_fn = jax.jit(
        lambda: tuple(jnp.zeros((NCORES * a.shape[0], *a.shape[1:]), a.dtype)
                      for a in out_avals),
        out_shardings=(zsh,) * n_outs)
    _cache["runner"] = (sharded, zeros_fn, in_names, out_names, zsh)
    return _cache["runner"]


def kernel(x, wq, wk, wv, wo, attention_mask=None, **_ignored):
    import jax
    from concurrent.futures import ThreadPoolExecutor
    sharded, zeros_fn, in_names, out_names, zsh = _get_runner()
    zs = zeros_fn()  # async: device-side zero buffers materialize in background

    # upload order = pack speed: wk/wv pack fastest, so their puts start
    # filling the (serial) tunnel while the bigger tensors are packed.
    # All weight slices are halved over the batch-pair (bi=0 -> first
    # half, bi=1 -> second half); wq/wo are permuted so head h packs
    # with h+4 (pairs (i, i+4) == reshape(2,4,..).swapaxes trick).
    wk16 = np.asarray(wk, dtype=np.float16)
    wv16 = np.asarray(wv, dtype=np.float16)
    cwkh = np.empty((NCORES * 1024, 128), np.float16)
    cwvh = np.empty((NCORES * 1024, 128), np.float16)
    for c in range(NCORES):
        bi, g = c // 4, c % 4
        cwkh[c * 1024:(c + 1) * 1024] = wk16[1024 * bi:1024 * (bi + 1),
                                             128 * g:128 * (g + 1)]
        cwvh[c * 1024:(c + 1) * 1024] = wv16[1024 * bi:1024 * (bi + 1),
                                             128 * g:128 * (g + 1)]
    dev = {"wkh": jax.device_put(cwkh, zsh), "wvh": jax.device_put(cwvh, zsh)}

    wq16 = np.asarray(wq, dtype=np.float16)
    cwqh = np.empty((NCORES * 1024, 512), np.float16)
    for g in range(4):
        wq_g = wq16[:, 512 * g:512 * (g + 1)].reshape(D, 2, 4, DH)
        wq_g = wq_g.swapaxes(1, 2).reshape(D, 512)
        cwqh[g * 1024:(g + 1) * 1024] = wq_g[:1024]
        cwqh[(4 + g) * 1024:(5 + g) * 1024] = wq_g[1024:]
    dev["wqh"] = jax.device_put(cwqh, zsh)

    # x: int8 with per-(128-T-block, D-column) absmax scales, then the
    # transposed T-quarter for each core's gather position
    xf = np.asarray(x, dtype=np.float32)
    cxq8 = np.empty((NCORES * D, 512), np.int8)
    cxscl = np.empty((NCORES * 16, D), np.float16)
    for bi in range(B):
        xb = xf[bi].reshape(16, 128, D)
        mx = np.abs(xb).max(axis=1)                      # [16, D]
        np.maximum(mx, 1e-30, out=mx)
        scl16 = (mx * (1.0 / 127.0)).astype(np.float16)  # uploaded scales
        inv = 127.0 / mx
        q8 = np.round(xb * inv[:, None, :]).astype(np.int8).reshape(T, D)
        for g in range(4):
            c = bi * 4 + g
            cxq8[c * D:(c + 1) * D] = q8[512 * g:512 * (g + 1), :].T
            cxscl[c * 16:(c + 1) * 16] = scl16
    dev["xq8"] = jax.device_put(cxq8, zsh)
    dev["xscl"] = jax.device_put(cxscl, zsh)

    wo16 = np.asarray(wo, dtype=np.float16)
    cwoh = np.empty((NCORES * 256, D), np.float16)
    for g in range(4):
        wo_g = wo16[512 * g:512 * (g + 1), :].reshape(2, 4, DH, D)
        wo_g = wo_g.swapaxes(0, 1).reshape(512, D)
        cwoh[g * 256:(g + 1) * 256] = wo_g[:256]
        cwoh[(4 + g) * 256:(5 + g) * 256] = wo_g[256:]
    dev["woh"] = jax.device_put(cwoh, zsh)

    out_arrs = sharded(*[dev[n] for n in in_names], *zs)
    # core order: (b0 q0..q3, b1 q0..q3), each [512, D] -> [B, T, D]
    with ThreadPoolExecutor(2) as ex:
        futs = [ex.submit(np.asarray, a) for a in out_arrs]
        outs = {n: f.result() for n, f in zip(out_names, futs)}
    scl = outs["oscl"].astype(np.float32) * (1.0 / 127.0)
    y = outs["out8"].astype(np.float32) * scl[:, None]
    return y.reshape(B, T, D)
